# revision 1
# baseline (speedup 1.0000x reference)
"""Trainium2 Bass kernel for nn_LlamaDLODecoderLayer (moe_routing).

Sharding: 8 cores = 4 batch rows x 2 query-halves. Each core processes the
512-query-token half of one row's K=1024 routed tokens; K/V are recomputed
for the full row on both cores of a pair (keeps the program SPMD-uniform,
no collectives). Host does routing gather/scatter, RoPE tables, weight
pre-tiling and bf16 cast; device does the full decoder layer in bf16
matmuls with fp32 softmax/residuals.
"""

import sys

sys.path.insert(0, "/opt/trn_rl_repo")

import math
from contextlib import ExitStack

import ml_dtypes
import numpy as np

import concourse.bacc as bacc
import concourse.bass as bass
import concourse.mybir as mybir
import concourse.tile as tile
from concourse.bass_utils import run_bass_kernel_spmd
from concourse.masks import make_identity

B, S, H, NH, DH, DFF = 4, 8192, 2048, 16, 128, 8192
K = 1024
TQ = 512          # query tokens per core
TKV = 1024        # kv tokens per core (full row)
EPS = 1e-5
THETA = 10000.0
NEG = -1e9

F32 = mybir.dt.float32
BF16 = mybir.dt.bfloat16
AF = mybir.ActivationFunctionType

NKT = H // 128    # 16 k-tiles over H
NFT = DFF // 128  # 64 f-tiles over DFF

_COMPILED = None


class St:
    """Shared build state."""
    pass


def _consts(st):
    nc, tc, ep = st.nc, st.tc, st.ep
    consts = ep(tc.tile_pool(name="consts", bufs=1))
    st.ident = consts.tile([128, 128], BF16)
    make_identity(nc, st.ident)
    st.cosq = consts.tile([DH, TQ], F32)
    st.sinq = consts.tile([DH, TQ], F32)
    st.cosk = consts.tile([DH, TKV], F32)
    st.sink = consts.tile([DH, TKV], F32)
    st.scale_sb = consts.tile([128, 1], F32)
    st.eps_sb = consts.tile([128, 1], F32)
    nc.vector.memset(st.eps_sb[:], EPS)
    nc.sync.dma_start(st.cosq[:], st.d["cosq"][:])
    nc.sync.dma_start(st.sinq[:], st.d["sinq"][:])
    nc.sync.dma_start(st.cosk[:], st.d["cosk"][:])
    nc.sync.dma_start(st.sink[:], st.d["sink"][:])
    nc.sync.dma_start(st.scale_sb[:], st.d["scale"][:])

    svals = ep(tc.tile_pool(name="svals", bufs=1))
    st.s_kv = svals.tile([128, 8], F32)
    st.s_q = svals.tile([128, 4], F32)
    st.s_2 = svals.tile([128, 4], F32)
    st.stmp = svals.tile([128, 32], F32)

    st.oT_pool = ep(tc.tile_pool(name="oT", bufs=16))


def _inv_rms(st, src_ap, dst_col, scratch, idx):
    nc = st.nc
    c0, c1 = 2 * idx, 2 * idx + 1
    nc.scalar.activation(scratch[:], src_ap, AF.Square,
                         accum_out=st.stmp[:, c0:c0 + 1])
    nc.scalar.activation(st.stmp[:, c1:c1 + 1], st.stmp[:, c0:c0 + 1], AF.Sqrt,
                         bias=st.eps_sb[:, 0:1], scale=1.0 / H)
    nc.vector.reciprocal(dst_col, st.stmp[:, c1:c1 + 1])


def _transpose_into(st, pool, dst_ap, src_ap):
    """PE-transpose a [128,128] bf16 block into dst via PSUM."""
    nc = st.nc
    tp = pool.tile([128, 128], BF16, tag="tp", name="tp")
    nc.tensor.transpose(tp[:], src_ap, st.ident[:])
    nc.vector.tensor_copy(dst_ap, tp[:])


def _stage_norm1(st, s1):
    """xn -> transposed xnT [H,TKV] and xnqT [H,TQ] (bf16)."""
    nc, tc = st.nc, st.tc
    xq_pool = s1.enter_context(tc.tile_pool(name="xq1", bufs=2))
    xload = s1.enter_context(tc.tile_pool(name="xload", bufs=2))
    sqscr = s1.enter_context(tc.tile_pool(name="sqscr", bufs=1))
    xn_pool = s1.enter_context(tc.tile_pool(name="xn", bufs=2))
    xnq_pool = s1.enter_context(tc.tile_pool(name="xnq", bufs=2))

    st.xnT = []
    for i in range(NKT):
        xi = st.xnT_pool.tile([128, TKV], BF16, tag="xnT", name="xnTt")
        st.xnT.append(xi)
    st.xnqT = []
    for i in range(NKT):
        xi = st.xnqT_pool.tile([128, TQ], BF16, tag="xnqT", name="xnqTt")
        st.xnqT.append(xi)

    for j in range(8):
        ld = xload.tile([128, H], BF16, tag="xload")
        nc.sync.dma_start(ld[:], st.d["xkv"][j * 128:(j + 1) * 128, :])
        scr = sqscr.tile([128, H], F32, tag="sq")
        _inv_rms(st, ld[:], st.s_kv[:, j:j + 1], scr, j)
        xn_j = xn_pool.tile([128, H], BF16, tag="xn")
        nc.vector.tensor_scalar_mul(xn_j[:], ld[:], st.s_kv[:, j:j + 1])
        for i in range(NKT):
            _transpose_into(st, st.tpsum, st.xnT[i][:, j * 128:(j + 1) * 128],
                            xn_j[:, i * 128:(i + 1) * 128])
    for t in range(4):
        xq_t = xq_pool.tile([128, H], F32, tag="xq1")
        nc.sync.dma_start(xq_t[:], st.d["xq"][t * 128:(t + 1) * 128, :])
        scr = sqscr.tile([128, H], F32, tag="sq")
        _inv_rms(st, xq_t[:], st.s_q[:, t:t + 1], scr, 8 + t)
        xnq_t = xnq_pool.tile([128, H], BF16, tag="xnq")
        nc.vector.tensor_scalar_mul(xnq_t[:], xq_t[:], st.s_q[:, t:t + 1])
        for i in range(NKT):
            _transpose_into(st, st.tpsum, st.xnqT[i][:, t * 128:(t + 1) * 128],
                            xnq_t[:, i * 128:(i + 1) * 128])


def _stage_v(st, s2):
    nc, tc = st.nc, st.tc
    wv_pool = s2.enter_context(tc.tile_pool(name="wv", bufs=16))
    vps = s2.enter_context(tc.tile_pool(name="vps", bufs=2, space="PSUM"))
    wv_sb = []
    for k in range(NKT):
        wt = wv_pool.tile([128, H], BF16, tag="wv")
        nc.sync.dma_start(wt[:], st.d["wv"][k])
        wv_sb.append(wt)
    st.v_sb = []
    for j in range(8):
        vt = st.v_pool.tile([128, H], BF16, tag="v", name="vt")
        for n in range(4):
            ps = vps.tile([128, 512], F32, tag="vps")
            for k in range(NKT):
                nc.tensor.matmul(ps[:], st.xnT[k][:, j * 128:(j + 1) * 128],
                                 wv_sb[k][:, n * 512:(n + 1) * 512],
                                 start=(k == 0), stop=(k == NKT - 1))
            nc.vector.tensor_copy(vt[:, n * 512:(n + 1) * 512], ps[:])
        st.v_sb.append(vt)


def _rope(st, p, dst_ap, src_psum, cos_ap, sin_ap):
    nc = st.nc
    qf = p["ropes"].tile([128, 512], F32, tag="rpa", name="qf")
    nc.vector.tensor_copy(qf[:], src_psum)
    rot = p["ropes"].tile([128, 512], F32, tag="rpb", name="rot")
    nc.scalar.mul(rot[0:64, :], qf[64:128, :], -1.0)
    nc.scalar.copy(rot[64:128, :], qf[0:64, :])
    nc.vector.tensor_mul(rot[:], rot[:], sin_ap)
    nc.vector.tensor_mul(qf[:], qf[:], cos_ap)
    nc.vector.tensor_add(dst_ap, qf[:], rot[:])


def _attn_head(st, p, hh):
    nc = st.nc
    wq_sb = p["wqk"].tile([128, H], BF16, tag="wq", name="wqt")
    nc.sync.dma_start(wq_sb[:], st.d["wq"][hh])
    wk_sb = p["wqk"].tile([128, H], BF16, tag="wk", name="wkt")
    nc.sync.dma_start(wk_sb[:], st.d["wk"][hh])

    qr = p["qkr"].tile([128, TQ], BF16, tag="qr", name="qrt")
    kr = p["qkr"].tile([128, TKV], BF16, tag="kr", name="krt")

    qps = p["qkps"].tile([128, 512], F32, tag="qk", name="qpst")
    for k in range(NKT):
        nc.tensor.matmul(qps[:], wq_sb[:, k * 128:(k + 1) * 128], st.xnqT[k][:],
                         start=(k == 0), stop=(k == NKT - 1))
    _rope(st, p, qr[:], qps[:], st.cosq[:], st.sinq[:])

    for half in range(2):
        kps = p["qkps"].tile([128, 512], F32, tag="qk", name="kpst")
        for k in range(NKT):
            nc.tensor.matmul(kps[:], wk_sb[:, k * 128:(k + 1) * 128],
                             st.xnT[k][:, half * 512:(half + 1) * 512],
                             start=(k == 0), stop=(k == NKT - 1))
        _rope(st, p, kr[:, half * 512:(half + 1) * 512], kps[:],
              st.cosk[:, half * 512:(half + 1) * 512],
              st.sink[:, half * 512:(half + 1) * 512])

    aT = []
    for _ in range(8):
        aT_t = p["attnT"].tile([128, TQ], BF16, tag="attnT", name="aTt")
        aT.append(aT_t)
    for t in range(4):
        p0 = p["scps"].tile([128, 512], F32, tag="sc", name="p0t")
        nc.tensor.matmul(p0[:], qr[:, t * 128:(t + 1) * 128], kr[:, 0:512],
                         start=True, stop=True)
        p1 = p["scps"].tile([128, 512], F32, tag="sc", name="p1t")
        nc.tensor.matmul(p1[:], qr[:, t * 128:(t + 1) * 128], kr[:, 512:1024],
                         start=True, stop=True)
        sc = p["scsb"].tile([128, TKV], F32, tag="sc", name="sct")
        nc.vector.tensor_add(sc[:, 0:512], p0[:], st.mask_sb[t][:, 0:512])
        nc.vector.tensor_add(sc[:, 512:1024], p1[:], st.mask_sb[t][:, 512:1024])
        nm = p["smv"].tile([128, 3], F32, tag="smv", name="nmt")
        nc.vector.tensor_reduce(nm[:, 0:1], sc[:], axis=mybir.AxisListType.X,
                                op=mybir.AluOpType.max, negate=True)
        pr = p["scsb"].tile([128, TKV], BF16, tag="pr", name="prt")
        nc.scalar.activation(pr[:], sc[:], AF.Exp, bias=nm[:, 0:1],
                             accum_out=nm[:, 1:2])
        nc.vector.reciprocal(nm[:, 2:3], nm[:, 1:2])
        nc.vector.tensor_scalar_mul(pr[:], pr[:], nm[:, 2:3])
        for kk in range(8):
            _transpose_into(st, st.tpsum, aT[kk][:, t * 128:(t + 1) * 128],
                            pr[:, kk * 128:(kk + 1) * 128])

    ops = p["avps"].tile([128, TQ], F32, tag="av", name="opst")
    for kk in range(8):
        nc.tensor.matmul(ops[:], st.v_sb[kk][:, hh * 128:(hh + 1) * 128],
                         aT[kk][:], start=(kk == 0), stop=(kk == 7))
    oT_h = st.oT_pool.tile([128, TQ], BF16, tag="oT", name="oTt")
    nc.vector.tensor_copy(oT_h[:], ops[:])
    st.oT.append(oT_h)


def _stage_attn(st, s3):
    tc = st.tc
    p = {
        "wqk": s3.enter_context(tc.tile_pool(name="wqk", bufs=3)),
        "qkps": s3.enter_context(tc.tile_pool(name="qkps", bufs=3, space="PSUM")),
        "ropes": s3.enter_context(tc.tile_pool(name="ropes", bufs=4)),
        "qkr": s3.enter_context(tc.tile_pool(name="qkr", bufs=4)),
        "scps": s3.enter_context(tc.tile_pool(name="scps", bufs=2, space="PSUM")),
        "scsb": s3.enter_context(tc.tile_pool(name="scsb", bufs=3)),
        "smv": s3.enter_context(tc.tile_pool(name="smv", bufs=2)),
        "attnT": s3.enter_context(tc.tile_pool(name="attnT", bufs=12)),
        "avps": s3.enter_context(tc.tile_pool(name="avps", bufs=1, space="PSUM")),
    }
    st.oT = []
    for hh in range(NH):
        _attn_head(st, p, hh)


def _stage_wo(st, s4):
    nc, tc = st.nc, st.tc
    xn2_sb = []
    st.xn2_pool = s4.enter_context(tc.tile_pool(name="xn2", bufs=4))
    with ExitStack() as s4w:
        xq_pool = s4w.enter_context(tc.tile_pool(name="xq2", bufs=4))
        wo_pool = s4w.enter_context(tc.tile_pool(name="wo", bufs=3))
        sq2 = s4w.enter_context(tc.tile_pool(name="sq2", bufs=1))
        wops = s4w.enter_context(tc.tile_pool(name="wops", bufs=8, space="PSUM"))
        xq_sb = []
        for t in range(4):
            xt = xq_pool.tile([128, H], F32, tag="xq2", name="xqt")
            nc.sync.dma_start(xt[:], st.d["xq"][t * 128:(t + 1) * 128, :])
            xq_sb.append(xt)
        for tp2 in range(2):
            ps = []
            for _a in range(2):
                row = []
                for _b in range(4):
                    pst = wops.tile([128, 512], F32, tag="wops", name="wopst")
                    row.append(pst)
                ps.append(row)
            for k in range(16):
                wo_sb = wo_pool.tile([128, H], BF16, tag="wo", name="wot")
                nc.sync.dma_start(wo_sb[:], st.d["wo"][k])
                for t2 in range(2):
                    t = tp2 * 2 + t2
                    for n in range(4):
                        nc.tensor.matmul(ps[t2][n][:],
                                         st.oT[k][:, t * 128:(t + 1) * 128],
                                         wo_sb[:, n * 512:(n + 1) * 512],
                                         start=(k == 0), stop=(k == 15))
            for t2 in range(2):
                t = tp2 * 2 + t2
                hs_t = st.hs_pool.tile([128, H], F32, tag="hs", name="hst")
                for n in range(4):
                    nc.vector.tensor_add(hs_t[:, n * 512:(n + 1) * 512],
                                         ps[t2][n][:],
                                         xq_sb[t][:, n * 512:(n + 1) * 512])
                scr = sq2.tile([128, H], F32, tag="sq2", name="scrt")
                _inv_rms(st, hs_t[:], st.s_2[:, t:t + 1], scr, 12 + t)
                xn2_t = st.xn2_pool.tile([128, H], BF16, tag="xn2", name="xn2t")
                nc.vector.tensor_scalar_mul(xn2_t[:], hs_t[:], st.s_2[:, t:t + 1])
                st.hs_sb.append(hs_t)
                xn2_sb.append(xn2_t)
    # transposes (after wops PSUM freed)
    tp2pool = s4.enter_context(tc.tile_pool(name="tpsum2", bufs=2, space="PSUM"))
    st.xn2T = []
    for i in range(NKT):
        xi = st.xn2T_pool.tile([128, TQ], BF16, tag="xn2T", name="xn2Tt")
        for t in range(4):
            _transpose_into(st, tp2pool, xi[:, t * 128:(t + 1) * 128],
                            xn2_sb[t][:, i * 128:(i + 1) * 128])
        st.xn2T.append(xi)


def _stage_mlp_gu(st, s5):
    nc, tc = st.nc, st.tc
    wgu_pool = s5.enter_context(tc.tile_pool(name="wgu", bufs=6))
    gps_pool = s5.enter_context(tc.tile_pool(name="gps", bufs=2, space="PSUM"))
    ups_pool = s5.enter_context(tc.tile_pool(name="ups", bufs=2, space="PSUM"))
    gsc = s5.enter_context(tc.tile_pool(name="gsc", bufs=2))
    st.hT = []
    for f in range(NFT):
        wg_sb = wgu_pool.tile([128, H], BF16, tag="wg", name="wgt")
        nc.sync.dma_start(wg_sb[:], st.d["wg"][f])
        wu_sb = wgu_pool.tile([128, H], BF16, tag="wu", name="wut")
        nc.sync.dma_start(wu_sb[:], st.d["wu"][f])
        gps = gps_pool.tile([128, TQ], F32, tag="g", name="gpst")
        ups = ups_pool.tile([128, TQ], F32, tag="u", name="upst")
        for k in range(NKT):
            nc.tensor.matmul(gps[:], wg_sb[:, k * 128:(k + 1) * 128],
                             st.xn2T[k][:], start=(k == 0), stop=(k == NKT - 1))
        for k in range(NKT):
            nc.tensor.matmul(ups[:], wu_sb[:, k * 128:(k + 1) * 128],
                             st.xn2T[k][:], start=(k == 0), stop=(k == NKT - 1))
        gs = gsc.tile([128, TQ], F32, tag="gs", name="gst")
        nc.scalar.activation(gs[:], gps[:], AF.Silu)
        hT_f = st.hT_pool.tile([128, TQ], BF16, tag="hT", name="hTt")
        nc.vector.tensor_mul(hT_f[:], gs[:], ups[:])
        st.hT.append(hT_f)


def _stage_down(st, s6):
    nc, tc = st.nc, st.tc
    wd_pool = s6.enter_context(tc.tile_pool(name="wd", bufs=3))
    dnps = s6.enter_context(tc.tile_pool(name="dnps", bufs=8, space="PSUM"))
    fin = s6.enter_context(tc.tile_pool(name="fin", bufs=2))
    for nh in range(2):
        ps = []
        for _a in range(4):
            row = []
            for _b in range(2):
                pst = dnps.tile([128, 512], F32, tag="dn", name="dnt")
                row.append(pst)
            ps.append(row)
        for k in range(NFT):
            wd_sb = wd_pool.tile([128, 1024], BF16, tag="wd", name="wdt")
            nc.sync.dma_start(wd_sb[:], st.d["wd"][k][:, nh * 1024:(nh + 1) * 1024])
            for t in range(4):
                for n2 in range(2):
                    nc.tensor.matmul(ps[t][n2][:],
                                     st.hT[k][:, t * 128:(t + 1) * 128],
                                     wd_sb[:, n2 * 512:(n2 + 1) * 512],
                                     start=(k == 0), stop=(k == NFT - 1))
        for t in range(4):
            for n2 in range(2):
                col = nh * 1024 + n2 * 512
                ft = fin.tile([128, 512], F32, tag="fin", name="ft")
                nc.vector.tensor_scalar_mul(ft[:], ps[t][n2][:],
                                            st.scale_sb[:, 0:1])
                nc.vector.tensor_add(ft[:], ft[:],
                                     st.hs_sb[t][:, col:col + 512])
                nc.sync.dma_start(st.d["out"][t * 128:(t + 1) * 128, col:col + 512],
                                  ft[:])


def _build():
    nc = bacc.Bacc()
    st = St()
    st.nc = nc
    d = {}
    d["xq"] = nc.dram_tensor("xq", [TQ, H], F32, kind="ExternalInput")
    d["xkv"] = nc.dram_tensor("xkv", [TKV, H], BF16, kind="ExternalInput")
    d["cosq"] = nc.dram_tensor("cosq", [DH, TQ], F32, kind="ExternalInput")
    d["sinq"] = nc.dram_tensor("sinq", [DH, TQ], F32, kind="ExternalInput")
    d["cosk"] = nc.dram_tensor("cosk", [DH, TKV], F32, kind="ExternalInput")
    d["sink"] = nc.dram_tensor("sink", [DH, TKV], F32, kind="ExternalInput")
    d["mask"] = nc.dram_tensor("mask", [TQ, TKV], F32, kind="ExternalInput")
    d["scale"] = nc.dram_tensor("scale", [128, 1], F32, kind="ExternalInput")
    d["wq"] = nc.dram_tensor("wq", [NH, 128, H], BF16, kind="ExternalInput")
    d["wk"] = nc.dram_tensor("wk", [NH, 128, H], BF16, kind="ExternalInput")
    d["wv"] = nc.dram_tensor("wv", [16, 128, H], BF16, kind="ExternalInput")
    d["wo"] = nc.dram_tensor("wo", [16, 128, H], BF16, kind="ExternalInput")
    d["wg"] = nc.dram_tensor("wg", [64, 128, H], BF16, kind="ExternalInput")
    d["wu"] = nc.dram_tensor("wu", [64, 128, H], BF16, kind="ExternalInput")
    d["wd"] = nc.dram_tensor("wd", [64, 128, H], BF16, kind="ExternalInput")
    d["out"] = nc.dram_tensor("out", [TQ, H], F32, kind="ExternalOutput")
    st.d = d

    with tile.TileContext(nc) as tc, ExitStack() as ctx:
        st.tc = tc
        st.ep = ctx.enter_context
        _consts(st)
        with ExitStack() as s123:
            e = s123.enter_context
            st.tpsum = e(tc.tile_pool(name="tpsum", bufs=2, space="PSUM"))
            mask_pool = e(tc.tile_pool(name="maskp", bufs=4))
            st.mask_sb = []
            for t in range(4):
                mt = mask_pool.tile([128, TKV], F32, tag="mask", name="mt")
                nc.sync.dma_start(mt[:], d["mask"][t * 128:(t + 1) * 128, :])
                st.mask_sb.append(mt)
            st.xnT_pool = e(tc.tile_pool(name="xnT", bufs=16))
            st.xnqT_pool = e(tc.tile_pool(name="xnqT", bufs=16))
            st.v_pool = e(tc.tile_pool(name="vsb", bufs=8))
            with ExitStack() as s1:
                _stage_norm1(st, s1)
            with ExitStack() as s2:
                _stage_v(st, s2)
            with ExitStack() as s3:
                _stage_attn(st, s3)
        with ExitStack() as s4567:
            e = s4567.enter_context
            st.hs_pool = e(tc.tile_pool(name="hs", bufs=4))
            st.xn2T_pool = e(tc.tile_pool(name="xn2T", bufs=16))
            st.hs_sb = []
            with ExitStack() as s4:
                _stage_wo(st, s4)
            with ExitStack() as s56:
                st.hT_pool = s56.enter_context(tc.tile_pool(name="hT", bufs=64))
                with ExitStack() as s5:
                    _stage_mlp_gu(st, s5)
                with ExitStack() as s6:
                    _stage_down(st, s6)

    nc.compile()
    return nc


def _prep_host(hidden_states, position_ids, topk_mask, topk_scores,
               Wq, Wk, Wv, Wo, Wgate, Wup, Wdown, ln1_w, ln2_w):
    bf16 = ml_dtypes.bfloat16
    order = np.argsort(np.where(topk_mask, 0, 1).astype(np.int32),
                       axis=1, kind="stable")
    topk_idx = order[:, :K]                                    # [B,K]
    bidx = np.arange(B)[:, None]
    x = np.ascontiguousarray(hidden_states[bidx, topk_idx])    # [B,K,H] f32
    pos = position_ids[bidx, topk_idx].astype(np.float32)      # [B,K]

    inv_freq = (1.0 / (THETA ** (np.arange(0, DH, 2, dtype=np.float32) / DH))
                ).astype(np.float32)
    freqs = pos[..., None] * inv_freq                          # [B,K,64]
    emb = np.concatenate([freqs, freqs], axis=-1)              # [B,K,128]
    cosT = np.ascontiguousarray(np.cos(emb).astype(np.float32).transpose(0, 2, 1))
    sinT = np.ascontiguousarray(np.sin(emb).astype(np.float32).transpose(0, 2, 1))

    l1 = ln1_w.astype(np.float32)[:, None]
    l2 = ln2_w.astype(np.float32)[:, None]
    wq_t = np.ascontiguousarray(
        (Wq * l1 / math.sqrt(DH)).reshape(16, 128, 16, 128)
        .transpose(2, 1, 0, 3).reshape(16, 128, H)).astype(bf16)
    wk_t = np.ascontiguousarray(
        (Wk * l1).reshape(16, 128, 16, 128)
        .transpose(2, 1, 0, 3).reshape(16, 128, H)).astype(bf16)
    wv_t = np.ascontiguousarray((Wv * l1).reshape(16, 128, H)).astype(bf16)
    wo_t = np.ascontiguousarray(Wo.reshape(16, 128, H)).astype(bf16)
    wg_t = np.ascontiguousarray(
        (Wgate * l2).reshape(16, 128, 64, 128)
        .transpose(2, 1, 0, 3).reshape(64, 128, H)).astype(bf16)
    wu_t = np.ascontiguousarray(
        (Wup * l2).reshape(16, 128, 64, 128)
        .transpose(2, 1, 0, 3).reshape(64, 128, H)).astype(bf16)
    wd_t = np.ascontiguousarray(Wdown.reshape(64, 128, H)).astype(bf16)

    qi = np.arange(TQ, dtype=np.int64)
    kj = np.arange(TKV, dtype=np.int64)

    in_maps = []
    for c in range(8):
        b, h = c // 2, c % 2
        q0 = h * TQ
        mask = np.where(kj[None, :] <= (q0 + qi)[:, None],
                        np.float32(0.0), np.float32(NEG)).astype(np.float32)
        scale_val = np.float32(0.5 * 1.0 + (topk_scores[b] - 0.5) * 1.0)
        in_maps.append({
            "xq": np.ascontiguousarray(x[b, q0:q0 + TQ]).astype(np.float32),
            "xkv": x[b].astype(bf16),
            "cosq": np.ascontiguousarray(cosT[b][:, q0:q0 + TQ]),
            "sinq": np.ascontiguousarray(sinT[b][:, q0:q0 + TQ]),
            "cosk": cosT[b],
            "sink": sinT[b],
            "mask": mask,
            "scale": np.full((128, 1), scale_val, dtype=np.float32),
            "wq": wq_t, "wk": wk_t, "wv": wv_t, "wo": wo_t,
            "wg": wg_t, "wu": wu_t, "wd": wd_t,
        })
    return in_maps, topk_idx, x


def kernel(hidden_states, position_ids, topk_mask, topk_scores, topk_k,
           Wq, Wk, Wv, Wo, Wgate, Wup, Wdown, ln1_w, ln2_w,
           _want_trace=False):
    global _COMPILED
    assert int(topk_k) == K
    hidden_states = np.asarray(hidden_states, dtype=np.float32)
    in_maps, topk_idx, _ = _prep_host(
        hidden_states, np.asarray(position_ids),
        np.asarray(topk_mask), np.asarray(topk_scores, dtype=np.float32),
        np.asarray(Wq, dtype=np.float32), np.asarray(Wk, dtype=np.float32),
        np.asarray(Wv, dtype=np.float32), np.asarray(Wo, dtype=np.float32),
        np.asarray(Wgate, dtype=np.float32), np.asarray(Wup, dtype=np.float32),
        np.asarray(Wdown, dtype=np.float32),
        np.asarray(ln1_w, dtype=np.float32), np.asarray(ln2_w, dtype=np.float32))

    if _COMPILED is None:
        _COMPILED = _build()
    nc = _COMPILED

    res = run_bass_kernel_spmd(nc, in_maps, list(range(8)), trace=_want_trace)
    kernel.last_exec_time_ns = res.exec_time_ns
    kernel.last_trace = res.instructions_and_trace

    out = hidden_states.copy()
    for c in range(8):
        b, h = c // 2, c % 2
        q0 = h * TQ
        out[b, topk_idx[b, q0:q0 + TQ]] = res.results[c]["out"]
    return out


kernel.last_exec_time_ns = None
kernel.last_trace = None

